# revision 29
# baseline (speedup 1.0000x reference)
"""Trainium2 Bass kernel for nn_BiomechanicsLoss (masked quadratic-form loss).

Math (per point): et = [u0, v1, w2, .5(u1+v0), .5(u2+w0), .5(w1+v2)],
q = et^T C et with C = inv(compliance) cast to f32.  Loss =
sqrt(sum_masked(q^2)) / count_masked, mask = gt_sdf < 1e-8.

Because q = et^T C et == et^T sym(C) et and C is block-diagonal
(3x3 normal block + diagonal shear block), with F = diag(1,1,1,.5,.5,.5):
  q = w11*s1^2 + w22*s2^2 + w33*s3^2 + w12*s1*s2 + w13*s1*s3 + w23*s2*s3
      + d*(s4^2 + s5^2 + s6^2)
where s1..s3 = u0, v1, w2 ; s4 = u1+v0 ; s5 = u2+w0 ; s6 = w1+v2 and the
weights come from M = F*sym(C)*F (all positive for these constants).

Sharding: pure data-parallel over the N point dimension across 8 cores.
Each core reduces its shard to per-partition partials [128, 2T]
(T per-tile sum(mask*q^2) columns + T count columns); host does the final
tiny reduction, sqrt and divide.

Engine split per [128, F] tile (F=1024 free elems/partition):
  VectorE: 3 f32 adds (shear), 3 fused weighted products (tensor_tensor_reduce
           scale), 1 mask compare, 8 bf16 combine adds (2x mode), 1 bf16 q*m
  ScalarE: 6 weighted squares via activation(Square, scale=sqrt(w)),
           Square(q*m) with accum_out -> sum(mask*q^2), Identity(m) with
           accum_out -> count
  DMA:     contiguous [128, F, 3] tiles (12KB/partition) via HWDGE
"""

import numpy as np

N = 4_194_304
NCORES = 8
N_LOCAL = N // NCORES  # 524288
P = 128
J = N_LOCAL // P  # 4096 points per partition (partition-major layout)
# chunk widths; tapered tail so the final serial compute chain is short
CHUNKS = [1024, 1024, 1024, 512, 512]
NT = len(CHUNKS)
assert sum(CHUNKS) == J

THRESH = 1e-8


def _weights():
    vp, Ep = 0.4, 0.21
    Ci = np.zeros((6, 6), dtype=np.float64)
    Ci[0, 0] = 1 / Ep;  Ci[0, 1] = -vp / Ep; Ci[0, 2] = -vp / Ep
    Ci[1, 0] = -vp / Ep; Ci[1, 1] = 1 / Ep;  Ci[1, 2] = -vp / Ep
    Ci[2, 0] = -vp;      Ci[2, 1] = -vp;     Ci[2, 2] = 1 / Ep
    Ci[3, 3] = 2 * (1 + vp) / Ep
    Ci[4, 4] = Ci[3, 3]
    Ci[5, 5] = Ci[3, 3]
    # match reference: inverse computed in f64, cast to f32
    C = np.linalg.inv(Ci).astype(np.float32).astype(np.float64)
    Cs = 0.5 * (C + C.T)
    A = Cs[:3, :3]
    d = 0.25 * Cs[3, 3]
    return dict(
        w11=A[0, 0], w22=A[1, 1], w33=A[2, 2],
        w12=2 * A[0, 1], w13=2 * A[0, 2], w23=2 * A[1, 2],
        d=d,
    )


_NC = None


def _build_nc():
    import concourse.bacc as bacc
    import concourse.mybir as mybir
    import concourse.tile as tile

    W = _weights()
    r11 = float(np.sqrt(W["w11"]))
    r22 = float(np.sqrt(W["w22"]))
    r33 = float(np.sqrt(W["w33"]))
    rd = float(np.sqrt(W["d"]))
    # factor cross weights: w12 = a1*a2, w13 = a1*a3, w23 = a2*a3 so the
    # products use pre-scaled bf16 copies (all bf16 -> DVE 2x mode)
    a1s = float(np.sqrt(W["w12"] * W["w13"] / W["w23"]))
    a2s = float(W["w12"] / a1s)
    a3s = float(W["w13"] / a1s)

    f32 = mybir.dt.float32
    bf16 = mybir.dt.bfloat16
    Sq = mybir.ActivationFunctionType.Square
    Ident = mybir.ActivationFunctionType.Identity
    ALU = mybir.AluOpType

    nc = bacc.Bacc()
    gu = nc.dram_tensor("gu", [N_LOCAL, 3], f32, kind="ExternalInput")
    gv = nc.dram_tensor("gv", [N_LOCAL, 3], f32, kind="ExternalInput")
    gw = nc.dram_tensor("gw", [N_LOCAL, 3], f32, kind="ExternalInput")
    sdf = nc.dram_tensor("sdf", [N_LOCAL], f32, kind="ExternalInput")
    out = nc.dram_tensor("out", [P, 2 * NT], f32, kind="ExternalOutput")

    # partition-major: partition p owns points [p*J, (p+1)*J) -- per-partition
    # DRAM runs are contiguous 48KB, chunks slice the free axis
    gu_r = gu[:, :].rearrange("(p j) c -> p j c", p=P)
    gv_r = gv[:, :].rearrange("(p j) c -> p j c", p=P)
    gw_r = gw[:, :].rearrange("(p j) c -> p j c", p=P)
    sdf_r = sdf[:].rearrange("(p j) -> p j", p=P)

    with tile.TileContext(nc) as tc:
        with (
            tc.tile_pool(name="iouv", bufs=3) as iouv,
            tc.tile_pool(name="iow", bufs=2) as iow,
            tc.tile_pool(name="mid", bufs=2) as mid,
            tc.tile_pool(name="stats", bufs=1) as stats_pool,
        ):
            stats = stats_pool.tile([P, 2 * NT], f32)
            # whole sdf shard in one 2MB transfer (avoids 5 small DMAs)
            sd_all = stats_pool.tile([P, J], f32)
            nc.sync.dma_start(out=sd_all[:], in_=sdf_r[:, :])

            off = 0
            for t, F in enumerate(CHUNKS):
                u = iouv.tile([P, F, 3], f32, tag="u")
                v = iouv.tile([P, F, 3], f32, tag="v")
                w = iow.tile([P, F, 3], f32, tag="w")
                sd = sd_all[:, off:off + F]
                nc.sync.dma_start(out=u[:], in_=gu_r[:, off:off + F, :])
                nc.sync.dma_start(out=v[:], in_=gv_r[:, off:off + F, :])
                nc.sync.dma_start(out=w[:], in_=gw_r[:, off:off + F, :])
                off += F

                u0, u1, u2 = u[:, :, 0], u[:, :, 1], u[:, :, 2]
                v0, v1, v2 = v[:, :, 0], v[:, :, 1], v[:, :, 2]
                w0, w1, w2 = w[:, :, 0], w[:, :, 1], w[:, :, 2]

                # shear strain components (f32 in, bf16 out; 1x)
                s4 = mid.tile([P, F], bf16, tag="s4")
                s5 = mid.tile([P, F], bf16, tag="s5")
                s6 = mid.tile([P, F], bf16, tag="s6")
                nc.vector.tensor_add(s4, u1, v0)
                nc.vector.tensor_add(s5, u2, w0)
                nc.vector.tensor_add(s6, w1, v2)

                # pre-scaled bf16 copies on ScalarE for the cross products;
                # the diagonal squares also read these
                p1 = mid.tile([P, F], bf16, tag="p1")
                p2 = mid.tile([P, F], bf16, tag="p2")
                p3 = mid.tile([P, F], bf16, tag="p3")
                nc.scalar.mul(p1, u0, a1s)
                nc.scalar.mul(p2, v1, a2s)
                nc.scalar.mul(p3, w2, a3s)

                # cross products, factored: p1*p2 + p1*p3 + p2*p3 =
                # p1*(p2+p3) + p2*p3 (all-bf16 -> DVE 2x mode)
                tp = mid.tile([P, F], bf16, tag="tp")
                ca = mid.tile([P, F], bf16, tag="ca")
                cb = mid.tile([P, F], bf16, tag="cb")
                nc.vector.tensor_add(tp, p2, p3)
                nc.vector.tensor_mul(ca, p1, tp)
                nc.vector.tensor_mul(cb, p2, p3)

                # mask (f32 single-src -> 2x mode); fused row-sum accum gives
                # the masked-point count for free
                m = mid.tile([P, F], bf16, tag="m")
                nc.vector.tensor_scalar(
                    out=m, in0=sd, scalar1=THRESH, scalar2=None, op0=ALU.is_lt,
                    op1=ALU.add, accum_out=stats[:, NT + t:NT + t + 1])

                # weighted squares on ScalarE: z = (sqrt(w)*x)^2
                z1 = mid.tile([P, F], bf16, tag="z1")
                z2 = mid.tile([P, F], bf16, tag="z2")
                z3 = mid.tile([P, F], bf16, tag="z3")
                z4 = mid.tile([P, F], bf16, tag="z4")
                z5 = mid.tile([P, F], bf16, tag="z5")
                z6 = mid.tile([P, F], bf16, tag="z6")
                nc.scalar.activation(z1, p1, Sq, scale=r11 / a1s)
                nc.scalar.activation(z2, p2, Sq, scale=r22 / a2s)
                nc.scalar.activation(z3, p3, Sq, scale=r33 / a3s)
                nc.scalar.activation(z4, s4, Sq, scale=rd)
                nc.scalar.activation(z5, s5, Sq, scale=rd)
                nc.scalar.activation(z6, s6, Sq, scale=rd)

                # combine: q = sum of 8 terms (bf16 2x adds, in place to keep
                # SBUF pressure down); q ends up in z1
                nc.vector.tensor_add(z1, z1, z2)
                nc.vector.tensor_add(z3, z3, ca)
                nc.vector.tensor_add(z4, z4, z5)
                nc.vector.tensor_add(cb, cb, z6)
                nc.vector.tensor_add(z1, z1, z3)
                nc.vector.tensor_add(z4, z4, cb)
                nc.vector.tensor_add(z1, z1, z4)

                # qm = q * mask (bf16 2x)
                qm = mid.tile([P, F], bf16, tag="qm")
                nc.vector.tensor_mul(qm, z1, m)

                # ssq_t = sum(qm^2) on ScalarE with fused row-sum accumulate
                junk1 = mid.tile([P, F], bf16, tag="junk1")
                nc.scalar.activation(
                    junk1, qm, Sq, accum_out=stats[:, t:t + 1])

            nc.sync.dma_start(out=out[:, :], in_=stats[:])

    nc.compile()
    return nc


def _get_nc():
    global _NC
    if _NC is None:
        _NC = _build_nc()
    return _NC


def _run(in_maps, trace=False, **kwargs):
    from concourse.bass_utils import run_bass_kernel_spmd

    nc = _get_nc()
    return run_bass_kernel_spmd(
        nc, in_maps, core_ids=list(range(NCORES)), trace=trace, **kwargs)


def _make_in_maps(grad_u, grad_v, grad_w, gt_sdf):
    grad_u = np.ascontiguousarray(np.asarray(grad_u, dtype=np.float32))
    grad_v = np.ascontiguousarray(np.asarray(grad_v, dtype=np.float32))
    grad_w = np.ascontiguousarray(np.asarray(grad_w, dtype=np.float32))
    gt_sdf = np.ascontiguousarray(np.asarray(gt_sdf, dtype=np.float32))
    in_maps = []
    for c in range(NCORES):
        sl = slice(c * N_LOCAL, (c + 1) * N_LOCAL)
        in_maps.append({
            "gu": grad_u[sl], "gv": grad_v[sl],
            "gw": grad_w[sl], "sdf": gt_sdf[sl],
        })
    return in_maps


def _finalize(results):
    ssq = 0.0
    cnt = 0.0
    for res in results:
        st = np.asarray(res["out"], dtype=np.float64)
        ssq += st[:, :NT].sum()
        cnt += st[:, NT:].sum()
    Wv = np.sqrt(ssq)
    return np.float32(Wv / cnt)


def kernel(grad_u, grad_v, grad_w, gt_sdf):
    in_maps = _make_in_maps(grad_u, grad_v, grad_w, gt_sdf)
    res = _run(in_maps, trace=False)
    return _finalize(res.results)


# revision 30
# speedup vs baseline: 1.0162x; 1.0162x over previous
"""Trainium2 Bass kernel for nn_BiomechanicsLoss (masked quadratic-form loss).

Math (per point): et = [u0, v1, w2, .5(u1+v0), .5(u2+w0), .5(w1+v2)],
q = et^T C et with C = inv(compliance) cast to f32.  Loss =
sqrt(sum_masked(q^2)) / count_masked, mask = gt_sdf < 1e-8.

Because q = et^T C et == et^T sym(C) et and C is block-diagonal
(3x3 normal block + diagonal shear block), with F = diag(1,1,1,.5,.5,.5):
  q = w11*s1^2 + w22*s2^2 + w33*s3^2 + w12*s1*s2 + w13*s1*s3 + w23*s2*s3
      + d*(s4^2 + s5^2 + s6^2)
where s1..s3 = u0, v1, w2 ; s4 = u1+v0 ; s5 = u2+w0 ; s6 = w1+v2 and the
weights come from M = F*sym(C)*F (all positive for these constants).

Sharding: pure data-parallel over the N point dimension across 8 cores.
Each core reduces its shard to per-partition partials [128, 2T]
(T per-tile sum(mask*q^2) columns + T count columns); host does the final
tiny reduction, sqrt and divide.

Engine split per [128, F] tile (F=1024 free elems/partition):
  VectorE: 3 f32 adds (shear), 3 fused weighted products (tensor_tensor_reduce
           scale), 1 mask compare, 8 bf16 combine adds (2x mode), 1 bf16 q*m
  ScalarE: 6 weighted squares via activation(Square, scale=sqrt(w)),
           Square(q*m) with accum_out -> sum(mask*q^2), Identity(m) with
           accum_out -> count
  DMA:     contiguous [128, F, 3] tiles (12KB/partition) via HWDGE
"""

import numpy as np

N = 4_194_304
NCORES = 8
N_LOCAL = N // NCORES  # 524288
P = 128
J = N_LOCAL // P  # 4096 points per partition (partition-major layout)
# chunk widths; tapered tail so the final serial compute chain is short
CHUNKS = [1024, 1024, 1024, 512, 512]
NT = len(CHUNKS)
assert sum(CHUNKS) == J

THRESH = 1e-8


def _weights():
    vp, Ep = 0.4, 0.21
    Ci = np.zeros((6, 6), dtype=np.float64)
    Ci[0, 0] = 1 / Ep;  Ci[0, 1] = -vp / Ep; Ci[0, 2] = -vp / Ep
    Ci[1, 0] = -vp / Ep; Ci[1, 1] = 1 / Ep;  Ci[1, 2] = -vp / Ep
    Ci[2, 0] = -vp;      Ci[2, 1] = -vp;     Ci[2, 2] = 1 / Ep
    Ci[3, 3] = 2 * (1 + vp) / Ep
    Ci[4, 4] = Ci[3, 3]
    Ci[5, 5] = Ci[3, 3]
    # match reference: inverse computed in f64, cast to f32
    C = np.linalg.inv(Ci).astype(np.float32).astype(np.float64)
    Cs = 0.5 * (C + C.T)
    A = Cs[:3, :3]
    d = 0.25 * Cs[3, 3]
    return dict(
        w11=A[0, 0], w22=A[1, 1], w33=A[2, 2],
        w12=2 * A[0, 1], w13=2 * A[0, 2], w23=2 * A[1, 2],
        d=d,
    )


_NC = None


def _build_nc():
    import concourse.bacc as bacc
    import concourse.mybir as mybir
    import concourse.tile as tile

    W = _weights()
    r11 = float(np.sqrt(W["w11"]))
    r22 = float(np.sqrt(W["w22"]))
    r33 = float(np.sqrt(W["w33"]))
    rd = float(np.sqrt(W["d"]))
    # factor cross weights: w12 = a1*a2, w13 = a1*a3, w23 = a2*a3 so the
    # products use pre-scaled bf16 copies (all bf16 -> DVE 2x mode)
    a1s = float(np.sqrt(W["w12"] * W["w13"] / W["w23"]))
    a2s = float(W["w12"] / a1s)
    a3s = float(W["w13"] / a1s)

    f32 = mybir.dt.float32
    bf16 = mybir.dt.bfloat16
    Sq = mybir.ActivationFunctionType.Square
    Ident = mybir.ActivationFunctionType.Identity
    ALU = mybir.AluOpType

    nc = bacc.Bacc()
    gu = nc.dram_tensor("gu", [N_LOCAL, 3], f32, kind="ExternalInput")
    gv = nc.dram_tensor("gv", [N_LOCAL, 3], f32, kind="ExternalInput")
    gw = nc.dram_tensor("gw", [N_LOCAL, 3], f32, kind="ExternalInput")
    sdf = nc.dram_tensor("sdf", [N_LOCAL], f32, kind="ExternalInput")
    out = nc.dram_tensor("out", [P, 2 * NT], f32, kind="ExternalOutput")

    # partition-major: partition p owns points [p*J, (p+1)*J) -- per-partition
    # DRAM runs are contiguous 48KB, chunks slice the free axis
    gu_r = gu[:, :].rearrange("(p j) c -> p j c", p=P)
    gv_r = gv[:, :].rearrange("(p j) c -> p j c", p=P)
    gw_r = gw[:, :].rearrange("(p j) c -> p j c", p=P)
    sdf_r = sdf[:].rearrange("(p j) -> p j", p=P)

    with tile.TileContext(nc) as tc:
        with (
            tc.tile_pool(name="iouv", bufs=3) as iouv,
            tc.tile_pool(name="iow", bufs=2) as iow,
            tc.tile_pool(name="mid", bufs=2) as mid,
            tc.tile_pool(name="stats", bufs=1) as stats_pool,
        ):
            stats = stats_pool.tile([P, 2 * NT], f32)
            # whole sdf shard in one 2MB transfer (avoids 5 small DMAs);
            # issued after chunk 0's grads so it doesn't delay first compute
            sd_all = stats_pool.tile([P, J], f32)

            off = 0
            for t, F in enumerate(CHUNKS):
                u = iouv.tile([P, F, 3], f32, tag="u")
                v = iouv.tile([P, F, 3], f32, tag="v")
                w = iow.tile([P, F, 3], f32, tag="w")
                sd = sd_all[:, off:off + F]
                nc.sync.dma_start(out=u[:], in_=gu_r[:, off:off + F, :])
                nc.sync.dma_start(out=v[:], in_=gv_r[:, off:off + F, :])
                nc.sync.dma_start(out=w[:], in_=gw_r[:, off:off + F, :])
                if t == 0:
                    nc.sync.dma_start(out=sd_all[:], in_=sdf_r[:, :])
                off += F

                u0, u1, u2 = u[:, :, 0], u[:, :, 1], u[:, :, 2]
                v0, v1, v2 = v[:, :, 0], v[:, :, 1], v[:, :, 2]
                w0, w1, w2 = w[:, :, 0], w[:, :, 1], w[:, :, 2]

                # shear strain components (f32 in, bf16 out; 1x)
                s4 = mid.tile([P, F], bf16, tag="s4")
                s5 = mid.tile([P, F], bf16, tag="s5")
                s6 = mid.tile([P, F], bf16, tag="s6")
                nc.vector.tensor_add(s4, u1, v0)
                nc.vector.tensor_add(s5, u2, w0)
                nc.vector.tensor_add(s6, w1, v2)

                # pre-scaled bf16 copies on ScalarE for the cross products;
                # the diagonal squares also read these
                p1 = mid.tile([P, F], bf16, tag="p1")
                p2 = mid.tile([P, F], bf16, tag="p2")
                p3 = mid.tile([P, F], bf16, tag="p3")
                nc.scalar.mul(p1, u0, a1s)
                nc.scalar.mul(p2, v1, a2s)
                nc.scalar.mul(p3, w2, a3s)

                # cross products, factored: p1*p2 + p1*p3 + p2*p3 =
                # p1*(p2+p3) + p2*p3 (all-bf16 -> DVE 2x mode)
                tp = mid.tile([P, F], bf16, tag="tp")
                ca = mid.tile([P, F], bf16, tag="ca")
                cb = mid.tile([P, F], bf16, tag="cb")
                nc.vector.tensor_add(tp, p2, p3)
                nc.vector.tensor_mul(ca, p1, tp)
                nc.vector.tensor_mul(cb, p2, p3)

                # mask (f32 single-src -> 2x mode); fused row-sum accum gives
                # the masked-point count for free
                m = mid.tile([P, F], bf16, tag="m")
                nc.vector.tensor_scalar(
                    out=m, in0=sd, scalar1=THRESH, scalar2=None, op0=ALU.is_lt,
                    op1=ALU.add, accum_out=stats[:, NT + t:NT + t + 1])

                # weighted squares on ScalarE: z = (sqrt(w)*x)^2
                z1 = mid.tile([P, F], bf16, tag="z1")
                z2 = mid.tile([P, F], bf16, tag="z2")
                z3 = mid.tile([P, F], bf16, tag="z3")
                z4 = mid.tile([P, F], bf16, tag="z4")
                z5 = mid.tile([P, F], bf16, tag="z5")
                z6 = mid.tile([P, F], bf16, tag="z6")
                nc.scalar.activation(z1, p1, Sq, scale=r11 / a1s)
                nc.scalar.activation(z2, p2, Sq, scale=r22 / a2s)
                nc.scalar.activation(z3, p3, Sq, scale=r33 / a3s)
                nc.scalar.activation(z4, s4, Sq, scale=rd)
                nc.scalar.activation(z5, s5, Sq, scale=rd)
                nc.scalar.activation(z6, s6, Sq, scale=rd)

                # combine: q = sum of 8 terms (bf16 2x adds, in place to keep
                # SBUF pressure down); q ends up in z1
                nc.vector.tensor_add(z1, z1, z2)
                nc.vector.tensor_add(z3, z3, ca)
                nc.vector.tensor_add(z4, z4, z5)
                nc.vector.tensor_add(cb, cb, z6)
                nc.vector.tensor_add(z1, z1, z3)
                nc.vector.tensor_add(z4, z4, cb)
                nc.vector.tensor_add(z1, z1, z4)

                # qm = q * mask (bf16 2x)
                qm = mid.tile([P, F], bf16, tag="qm")
                nc.vector.tensor_mul(qm, z1, m)

                # ssq_t = sum(qm^2) on ScalarE with fused row-sum accumulate
                junk1 = mid.tile([P, F], bf16, tag="junk1")
                nc.scalar.activation(
                    junk1, qm, Sq, accum_out=stats[:, t:t + 1])

            nc.sync.dma_start(out=out[:, :], in_=stats[:])

    nc.compile()
    return nc


def _get_nc():
    global _NC
    if _NC is None:
        _NC = _build_nc()
    return _NC


def _run(in_maps, trace=False, **kwargs):
    from concourse.bass_utils import run_bass_kernel_spmd

    nc = _get_nc()
    return run_bass_kernel_spmd(
        nc, in_maps, core_ids=list(range(NCORES)), trace=trace, **kwargs)


def _make_in_maps(grad_u, grad_v, grad_w, gt_sdf):
    grad_u = np.ascontiguousarray(np.asarray(grad_u, dtype=np.float32))
    grad_v = np.ascontiguousarray(np.asarray(grad_v, dtype=np.float32))
    grad_w = np.ascontiguousarray(np.asarray(grad_w, dtype=np.float32))
    gt_sdf = np.ascontiguousarray(np.asarray(gt_sdf, dtype=np.float32))
    in_maps = []
    for c in range(NCORES):
        sl = slice(c * N_LOCAL, (c + 1) * N_LOCAL)
        in_maps.append({
            "gu": grad_u[sl], "gv": grad_v[sl],
            "gw": grad_w[sl], "sdf": gt_sdf[sl],
        })
    return in_maps


def _finalize(results):
    ssq = 0.0
    cnt = 0.0
    for res in results:
        st = np.asarray(res["out"], dtype=np.float64)
        ssq += st[:, :NT].sum()
        cnt += st[:, NT:].sum()
    Wv = np.sqrt(ssq)
    return np.float32(Wv / cnt)


def kernel(grad_u, grad_v, grad_w, gt_sdf):
    in_maps = _make_in_maps(grad_u, grad_v, grad_w, gt_sdf)
    res = _run(in_maps, trace=False)
    return _finalize(res.results)


# revision 31
# speedup vs baseline: 1.0427x; 1.0261x over previous
"""Trainium2 Bass kernel for nn_BiomechanicsLoss (masked quadratic-form loss).

Math (per point): et = [u0, v1, w2, .5(u1+v0), .5(u2+w0), .5(w1+v2)],
q = et^T C et with C = inv(compliance) cast to f32.  Loss =
sqrt(sum_masked(q^2)) / count_masked, mask = gt_sdf < 1e-8.

Because q = et^T C et == et^T sym(C) et and C is block-diagonal
(3x3 normal block + diagonal shear block), with F = diag(1,1,1,.5,.5,.5):
  q = w11*s1^2 + w22*s2^2 + w33*s3^2 + w12*s1*s2 + w13*s1*s3 + w23*s2*s3
      + d*(s4^2 + s5^2 + s6^2)
where s1..s3 = u0, v1, w2 ; s4 = u1+v0 ; s5 = u2+w0 ; s6 = w1+v2 and the
weights come from M = F*sym(C)*F (all positive for these constants).

Sharding: pure data-parallel over the N point dimension across 8 cores.
Each core reduces its shard to per-partition partials [128, 2T]
(T per-tile sum(mask*q^2) columns + T count columns); host does the final
tiny reduction, sqrt and divide.

Engine split per [128, F] tile (F=1024 free elems/partition):
  VectorE: 3 f32 adds (shear), 3 fused weighted products (tensor_tensor_reduce
           scale), 1 mask compare, 8 bf16 combine adds (2x mode), 1 bf16 q*m
  ScalarE: 6 weighted squares via activation(Square, scale=sqrt(w)),
           Square(q*m) with accum_out -> sum(mask*q^2), Identity(m) with
           accum_out -> count
  DMA:     contiguous [128, F, 3] tiles (12KB/partition) via HWDGE
"""

import numpy as np

N = 4_194_304
NCORES = 8
N_LOCAL = N // NCORES  # 524288
P = 128
J = N_LOCAL // P  # 4096 points per partition (partition-major layout)
# chunk widths; tapered tail so the final serial compute chain is short
CHUNKS = [1024, 1024, 1024, 512, 512]
NT = len(CHUNKS)
assert sum(CHUNKS) == J

THRESH = 1e-8


def _weights():
    vp, Ep = 0.4, 0.21
    Ci = np.zeros((6, 6), dtype=np.float64)
    Ci[0, 0] = 1 / Ep;  Ci[0, 1] = -vp / Ep; Ci[0, 2] = -vp / Ep
    Ci[1, 0] = -vp / Ep; Ci[1, 1] = 1 / Ep;  Ci[1, 2] = -vp / Ep
    Ci[2, 0] = -vp;      Ci[2, 1] = -vp;     Ci[2, 2] = 1 / Ep
    Ci[3, 3] = 2 * (1 + vp) / Ep
    Ci[4, 4] = Ci[3, 3]
    Ci[5, 5] = Ci[3, 3]
    # match reference: inverse computed in f64, cast to f32
    C = np.linalg.inv(Ci).astype(np.float32).astype(np.float64)
    Cs = 0.5 * (C + C.T)
    A = Cs[:3, :3]
    d = 0.25 * Cs[3, 3]
    return dict(
        w11=A[0, 0], w22=A[1, 1], w33=A[2, 2],
        w12=2 * A[0, 1], w13=2 * A[0, 2], w23=2 * A[1, 2],
        d=d,
    )


_NC = None


def _build_nc():
    import concourse.bacc as bacc
    import concourse.mybir as mybir
    import concourse.tile as tile

    W = _weights()
    r11 = float(np.sqrt(W["w11"]))
    r22 = float(np.sqrt(W["w22"]))
    r33 = float(np.sqrt(W["w33"]))
    rd = float(np.sqrt(W["d"]))
    # factor cross weights: w12 = a1*a2, w13 = a1*a3, w23 = a2*a3 so the
    # products use pre-scaled bf16 copies (all bf16 -> DVE 2x mode)
    a1s = float(np.sqrt(W["w12"] * W["w13"] / W["w23"]))
    a2s = float(W["w12"] / a1s)
    a3s = float(W["w13"] / a1s)

    f32 = mybir.dt.float32
    bf16 = mybir.dt.bfloat16
    Sq = mybir.ActivationFunctionType.Square
    Ident = mybir.ActivationFunctionType.Identity
    ALU = mybir.AluOpType

    nc = bacc.Bacc()
    gu = nc.dram_tensor("gu", [N_LOCAL, 3], f32, kind="ExternalInput")
    gv = nc.dram_tensor("gv", [N_LOCAL, 3], f32, kind="ExternalInput")
    gw = nc.dram_tensor("gw", [N_LOCAL, 3], f32, kind="ExternalInput")
    sdf = nc.dram_tensor("sdf", [N_LOCAL], f32, kind="ExternalInput")
    out = nc.dram_tensor("out", [P, 2 * NT], f32, kind="ExternalOutput")

    # partition-major: partition p owns points [p*J, (p+1)*J) -- per-partition
    # DRAM runs are contiguous 48KB, chunks slice the free axis
    gu_r = gu[:, :].rearrange("(p j) c -> p j c", p=P)
    gv_r = gv[:, :].rearrange("(p j) c -> p j c", p=P)
    gw_r = gw[:, :].rearrange("(p j) c -> p j c", p=P)
    sdf_r = sdf[:].rearrange("(p j) -> p j", p=P)

    with tile.TileContext(nc) as tc:
        with (
            tc.tile_pool(name="io", bufs=2) as io,
            tc.tile_pool(name="mid", bufs=3) as mid,
            tc.tile_pool(name="stats", bufs=1) as stats_pool,
        ):
            stats = stats_pool.tile([P, 2 * NT], f32)

            off = 0
            for t, F in enumerate(CHUNKS):
                u = io.tile([P, F, 3], f32, tag="u")
                v = io.tile([P, F, 3], f32, tag="v")
                w = io.tile([P, F, 3], f32, tag="w")
                sd = io.tile([P, F], f32, tag="sd")
                nc.sync.dma_start(out=u[:], in_=gu_r[:, off:off + F, :])
                nc.sync.dma_start(out=v[:], in_=gv_r[:, off:off + F, :])
                nc.sync.dma_start(out=w[:], in_=gw_r[:, off:off + F, :])
                nc.sync.dma_start(out=sd[:], in_=sdf_r[:, off:off + F])
                off += F

                u0, u1, u2 = u[:, :, 0], u[:, :, 1], u[:, :, 2]
                v0, v1, v2 = v[:, :, 0], v[:, :, 1], v[:, :, 2]
                w0, w1, w2 = w[:, :, 0], w[:, :, 1], w[:, :, 2]

                # shear strain components (f32 in, bf16 out; 1x)
                s4 = mid.tile([P, F], bf16, tag="s4")
                s5 = mid.tile([P, F], bf16, tag="s5")
                s6 = mid.tile([P, F], bf16, tag="s6")
                nc.vector.tensor_add(s4, u1, v0)
                nc.vector.tensor_add(s5, u2, w0)
                nc.vector.tensor_add(s6, w1, v2)

                # pre-scaled bf16 copies on ScalarE for the cross products;
                # the diagonal squares also read these
                p1 = mid.tile([P, F], bf16, tag="p1")
                p2 = mid.tile([P, F], bf16, tag="p2")
                p3 = mid.tile([P, F], bf16, tag="p3")
                nc.scalar.mul(p1, u0, a1s)
                nc.scalar.mul(p2, v1, a2s)
                nc.scalar.mul(p3, w2, a3s)

                # cross products, factored: p1*p2 + p1*p3 + p2*p3 =
                # p1*(p2+p3) + p2*p3 (all-bf16 -> DVE 2x mode)
                tp = mid.tile([P, F], bf16, tag="tp")
                ca = mid.tile([P, F], bf16, tag="ca")
                cb = mid.tile([P, F], bf16, tag="cb")
                nc.vector.tensor_add(tp, p2, p3)
                nc.vector.tensor_mul(ca, p1, tp)
                nc.vector.tensor_mul(cb, p2, p3)

                # mask (f32 single-src -> 2x mode); fused row-sum accum gives
                # the masked-point count for free
                m = mid.tile([P, F], bf16, tag="m")
                nc.vector.tensor_scalar(
                    out=m, in0=sd, scalar1=THRESH, scalar2=None, op0=ALU.is_lt,
                    op1=ALU.add, accum_out=stats[:, NT + t:NT + t + 1])

                # weighted squares on ScalarE: z = (sqrt(w)*x)^2
                z1 = mid.tile([P, F], bf16, tag="z1")
                z2 = mid.tile([P, F], bf16, tag="z2")
                z3 = mid.tile([P, F], bf16, tag="z3")
                z4 = mid.tile([P, F], bf16, tag="z4")
                z5 = mid.tile([P, F], bf16, tag="z5")
                z6 = mid.tile([P, F], bf16, tag="z6")
                nc.scalar.activation(z1, p1, Sq, scale=r11 / a1s)
                nc.scalar.activation(z2, p2, Sq, scale=r22 / a2s)
                nc.scalar.activation(z3, p3, Sq, scale=r33 / a3s)
                nc.scalar.activation(z4, s4, Sq, scale=rd)
                nc.scalar.activation(z5, s5, Sq, scale=rd)
                nc.scalar.activation(z6, s6, Sq, scale=rd)

                # combine: q = sum of 8 terms (bf16 2x adds, in place to keep
                # SBUF pressure down); q ends up in z1
                nc.vector.tensor_add(z1, z1, z2)
                nc.vector.tensor_add(z3, z3, ca)
                nc.vector.tensor_add(z4, z4, z5)
                nc.vector.tensor_add(cb, cb, z6)
                nc.vector.tensor_add(z1, z1, z3)
                nc.vector.tensor_add(z4, z4, cb)
                nc.vector.tensor_add(z1, z1, z4)

                # qm = q * mask (bf16 2x)
                qm = mid.tile([P, F], bf16, tag="qm")
                nc.vector.tensor_mul(qm, z1, m)

                # ssq_t = sum(qm^2) on ScalarE with fused row-sum accumulate
                junk1 = mid.tile([P, F], bf16, tag="junk1")
                nc.scalar.activation(
                    junk1, qm, Sq, accum_out=stats[:, t:t + 1])

            nc.sync.dma_start(out=out[:, :], in_=stats[:])

    nc.compile()
    return nc


def _get_nc():
    global _NC
    if _NC is None:
        _NC = _build_nc()
    return _NC


def _run(in_maps, trace=False, **kwargs):
    from concourse.bass_utils import run_bass_kernel_spmd

    nc = _get_nc()
    return run_bass_kernel_spmd(
        nc, in_maps, core_ids=list(range(NCORES)), trace=trace, **kwargs)


def _make_in_maps(grad_u, grad_v, grad_w, gt_sdf):
    grad_u = np.ascontiguousarray(np.asarray(grad_u, dtype=np.float32))
    grad_v = np.ascontiguousarray(np.asarray(grad_v, dtype=np.float32))
    grad_w = np.ascontiguousarray(np.asarray(grad_w, dtype=np.float32))
    gt_sdf = np.ascontiguousarray(np.asarray(gt_sdf, dtype=np.float32))
    in_maps = []
    for c in range(NCORES):
        sl = slice(c * N_LOCAL, (c + 1) * N_LOCAL)
        in_maps.append({
            "gu": grad_u[sl], "gv": grad_v[sl],
            "gw": grad_w[sl], "sdf": gt_sdf[sl],
        })
    return in_maps


def _finalize(results):
    ssq = 0.0
    cnt = 0.0
    for res in results:
        st = np.asarray(res["out"], dtype=np.float64)
        ssq += st[:, :NT].sum()
        cnt += st[:, NT:].sum()
    Wv = np.sqrt(ssq)
    return np.float32(Wv / cnt)


def kernel(grad_u, grad_v, grad_w, gt_sdf):
    in_maps = _make_in_maps(grad_u, grad_v, grad_w, gt_sdf)
    res = _run(in_maps, trace=False)
    return _finalize(res.results)
